# revision 1
# baseline (speedup 1.0000x reference)
"""Trainium2 Bass kernel for nn_Attention_44074954391673.

Sharding: 8 cores; core c -> batch b = c//4, heads [3*(c%4), 3*(c%4)+3).
All matmuls bf16 with fp32 PSUM accumulation.

Host-side scaling folds: Wq /= 8 (q k^T and q-side rel pre-scaled by
1/sqrt(dh)); k-side rel table E /= 8; ssan_w *= 8 (cancels q pre-scale).

Relative-position (Toeplitz) terms: windowed [128, 1152] matmuls against the
(reversed) distance-embedding table -> DRAM scratch -> diagonal-AP DMA
read-back.  Q-side reads back skewed rows directly (r1); K-side reads back
skewed [r, l] tiles (g) which the PE transposes into bf16 PSUM; both join the
score accumulation via identity-matmul adds.

Scores PSUM group: packed-K64 QK^T + Id@r1 + Id@t2 + Id@(2 struct products).
Remaining 3 struct products summed in a small bf16 tree, base folded in via
scalar_tensor_tensor against PSUM, exp on ScalarE with accum_out row-sums.
"""
import os
import sys

sys.path.insert(0, "/opt/trn_rl_repo")

from contextlib import ExitStack

import numpy as np
import ml_dtypes

import concourse.bass as bass
import concourse.mybir as mybir
import concourse.tile as tile
from concourse import bacc
from concourse.bass import ds
from concourse.bass_utils import run_bass_kernel_spmd

BF16 = mybir.dt.bfloat16
F32 = mybir.dt.float32
AF = mybir.ActivationFunctionType
OP = mybir.AluOpType

H, DH = 12, 64
B, L, D = 2, 1024, 768
NCHUNK = 6
WIN = 1152
NT = 8


def build_program(use_mask: bool, use_pbias: bool, reps: int = 1):
    nc = bacc.Bacc("TRN2", target_bir_lowering=False, debug=False, num_devices=8)

    def din(name, shape, dt=BF16):
        return nc.dram_tensor(name, shape, dt, kind="ExternalInput").ap()

    hsT = din("hsT", [128, NCHUNK, 1024])
    wq = din("wq", [128, NCHUNK, 192])
    wk = din("wk", [128, NCHUNK, 192])
    wv = din("wv", [128, NCHUNK, 192])
    ert = din("ert", [128, 2048])
    et = din("et", [128, 2048])
    ssw = din("ssw", [64, 5, 3, 64])
    struct = din("struct", [NT, 128, 5, 1024])
    absb = din("absb", [1, 16], F32)
    idb = din("idb", [128, 128])
    idf = din("idf", [128, 128], F32)
    if use_mask:
        maskv = din("maskv", [1, 1024])
        onesv = din("onesv", [1, 128])
    if use_pbias:
        bqv = din("bqv", [1, 192])
        bkv = din("bkv", [1, 192])
        bvv = din("bvv", [1, 192])
        onesL = din("onesL", [1, 1024])
    out = nc.dram_tensor("out", [NT, 128, 192], F32, kind="ExternalOutput").ap()

    with tile.TileContext(nc) as tc:
        for _rep in range(reps):
          with ExitStack() as ctx:
            # ---------------- constants ----------------
            consts = ctx.enter_context(tc.tile_pool(name="consts", bufs=1))

            def cload(ap_in, shape=None, dt=BF16, name=None):
                t = consts.tile(shape, dt, name=name)
                nc.sync.dma_start(t, ap_in)
                return t

            sb_hsT = cload(hsT, name="hsT", shape=[128, NCHUNK, 1024])
            sb_wq = cload(wq, name="wq", shape=[128, NCHUNK, 192])
            sb_wk = cload(wk, name="wk", shape=[128, NCHUNK, 192])
            sb_wv = cload(wv, name="wv", shape=[128, NCHUNK, 192])
            sb_ert = cload(ert, name="ert", shape=[128, 2048])
            sb_et = cload(et, name="et", shape=[128, 2048])
            sb_ssw = cload(ssw, name="ssw", shape=[64, 5, 3, 64])
            sb_absb = cload(
                bass.AP(tensor=absb.tensor, offset=0, ap=[[0, 128], [1, 16]]),
                name="absb_sb", shape=[128, 16], dt=F32,
            )
            sb_idb = cload(idb, name="idb", shape=[128, 128])
            sb_idf = cload(idf, name="idf", shape=[128, 128], dt=F32)
            if use_mask:
                sb_mask = cload(maskv, name="maskv", shape=[1, 1024])
                sb_ones = cload(onesv, name="onesv", shape=[1, 128])
            if use_pbias:
                sb_bq = cload(bqv, name="bqv", shape=[1, 192])
                sb_bk = cload(bkv, name="bkv", shape=[1, 192])
                sb_bv = cload(bvv, name="bvv", shape=[1, 192])
                sb_onesL = cload(onesL, name="onesL", shape=[1, 1024])

            qkv = ctx.enter_context(tc.tile_pool(name="qkv", bufs=1))
            qd = [qkv.tile([128, 1024], BF16, tag=f"qd{h}", name=f"qd{h}") for h in range(3)]
            kd = [qkv.tile([128, 1024], BF16, tag=f"kd{h}", name=f"kd{h}") for h in range(3)]
            vsb = qkv.tile([128, NT, 192], BF16, name="vsb")

            # ---------------- projections ----------------
            with tc.tile_pool(name="pp", bufs=2, space="PSUM") as pp, \
                 tc.tile_pool(name="ppb", bufs=2, space="PSUM") as ppb, \
                 tc.tile_pool(name="ptmp", bufs=1) as ptmp:
                vta = ptmp.tile([128, 1024], BF16, tag="vta")
                vtb = ptmp.tile([64, 1024], BF16, tag="vtb")

                def proj_mms(w_sb, bias_sb, mlo, msz, n):
                    ps = pp.tile([128, 512], F32, tag="proj")
                    for c in range(NCHUNK):
                        last = (c == NCHUNK - 1) and not use_pbias
                        nc.tensor.matmul(
                            ps[0:msz, :],
                            lhsT=w_sb[:, c, ds(mlo, msz)],
                            rhs=sb_hsT[:, c, ds(512 * n, 512)],
                            start=(c == 0), stop=last,
                        )
                    if use_pbias:
                        nc.tensor.matmul(
                            ps[0:msz, :],
                            lhsT=bias_sb[0:1, ds(mlo, msz)],
                            rhs=sb_onesL[0:1, ds(512 * n, 512)],
                            start=False, stop=True,
                        )
                    return ps

                for w_sb, bias_sb, dup in (
                    (sb_wq, (sb_bq if use_pbias else None), qd),
                    (sb_wk, (sb_bk if use_pbias else None), kd),
                ):
                    for n in range(2):
                        sl = ds(512 * n, 512)
                        ps = proj_mms(w_sb, bias_sb, 0, 128, n)
                        # heads 0 (psum rows 0-63) and 1 (rows 64-127);
                        # partition-aligned evacs, dup via DMA afterwards
                        nc.scalar.activation(dup[0][0:64, sl], ps[0:64, :], AF.Copy)
                        nc.scalar.activation(dup[1][64:128, sl], ps[64:128, :], AF.Copy)
                        ps = proj_mms(w_sb, bias_sb, 128, 64, n)
                        nc.scalar.activation(dup[2][0:64, sl], ps[0:64, :], AF.Copy)
                    for h, (src, dst) in enumerate(((0, 64), (64, 0), (0, 64))):
                        nc.sync.dma_start(
                            dup[h][dst:dst + 64, :], dup[h][src:src + 64, :]
                        )
                for n in range(2):
                    sl = ds(512 * n, 512)
                    ps = proj_mms(sb_wv, (sb_bv if use_pbias else None), 0, 128, n)
                    nc.scalar.activation(vta[:, sl], ps, AF.Copy)
                    ps = proj_mms(sb_wv, (sb_bv if use_pbias else None), 128, 64, n)
                    nc.scalar.activation(vtb[:, sl], ps[0:64, :], AF.Copy)
                for t in range(NT):
                    pst = ppb.tile([128, 128], BF16, tag="vtp")
                    nc.tensor.matmul(
                        pst, lhsT=vta[:, ds(128 * t, 128)], rhs=sb_idb,
                        is_transpose=True, start=True, stop=True,
                    )
                    nc.scalar.activation(vsb[:, t, 0:128], pst, AF.Copy)
                    pst2 = ppb.tile([128, 64], BF16, tag="vtp2")
                    nc.tensor.matmul(
                        pst2, lhsT=vtb[:, ds(128 * t, 128)], rhs=sb_idb[0:64, 0:64],
                        is_transpose=True, start=True, stop=True,
                    )
                    nc.scalar.activation(vsb[:, t, 128:192], pst2, AF.Copy)

            dpool = ctx.enter_context(tc.tile_pool(name="dscr", bufs=2, space="DRAM"))

            # ---------------- per (b,h) ----------------
            # Unified PSUM pools across all phases: pscore = 2-bank score tiles,
            # putil = 1-bank utility tiles (windowed rel chunks, bias, transposes,
            # qw chunks, ctx).  8 banks total; no pool-boundary serialization.
            psS = ctx.enter_context(tc.tile_pool(name="psS", bufs=1, space="PSUM"))
            putil = ctx.enter_context(tc.tile_pool(name="putil", bufs=6, space="PSUM"))
            sstp = ctx.enter_context(tc.tile_pool(name="sstp", bufs=3))
            rg = ctx.enter_context(tc.tile_pool(name="rg", bufs=3))
            wev = ctx.enter_context(tc.tile_pool(name="wev", bufs=3))
            bep = ctx.enter_context(tc.tile_pool(name="bep", bufs=1))
            prp = ctx.enter_context(tc.tile_pool(name="prp", bufs=3))
            prob = ctx.enter_context(tc.tile_pool(name="prob", bufs=3))
            misc = ctx.enter_context(tc.tile_pool(name="misc", bufs=3))
            qwp = ctx.enter_context(tc.tile_pool(name="qwp", bufs=2))

            for h in range(3):
                qT, kT = qd[h], kd[h]
                dramQ = dpool.tile([NT, 128, WIN], BF16, tag="dq", name=f"dq{h}")
                dramK = dpool.tile([NT, 128, WIN], BF16, tag="dk", name=f"dk{h}")

                # B1/B2: windowed rel matmuls -> scratch (packed K=64 pairs)
                for src, rhs_tab, dst, alt in (
                    (qT, sb_ert, dramQ, 0), (kT, sb_et, dramK, 1),
                ):
                    for t0 in range(0, NT, 2):
                        evs = []
                        for pi, tt in ((0, t0), (1, t0 + 1)):
                            ev = wev.tile([128, WIN], BF16, tag="wev", name="wev")
                            win = 896 - 128 * tt
                            for ci, (c0, w) in enumerate(((0, 512), (512, 512), (1024, 128))):
                                pw = putil.tile([128, 512], F32, tag="u", name="pw")
                                nc.tensor.matmul(
                                    pw[:, 0:w],
                                    lhsT=src[64 * pi:64 * pi + 64, ds(128 * tt, 128)],
                                    rhs=rhs_tab[64 * pi:64 * pi + 64, ds(win + c0, w)],
                                    start=True, stop=True, tile_position=(64 * pi, 0),
                                )
                                # evac chunk routing: KB12 = act|dve|alt|alt2
                                kb = "alt"
                                use_act = (kb == "act") or (kb == "alt" and (ci + pi + alt) % 2 == 0) or (kb == "alt2" and (ci + pi + alt) % 3 != 0)
                                if use_act:
                                    nc.scalar.activation(ev[:, ds(c0, w)], pw[:, 0:w], AF.Copy)
                                else:
                                    nc.vector.tensor_copy(ev[:, ds(c0, w)], pw[:, 0:w])
                            evs.append((ev, tt))
                        for ev, tt in evs:
                            nc.sync.dma_start(dst[tt], ev)

                # B3: qw_i (chunked into 1-bank psum tiles) + dup
                qw_sb = qwp.tile([128, 5, 1024], BF16, tag="qw", name="qw_sb")
                for i in range(5):
                    for n2 in range(2):
                        pq = putil.tile([64, 512], F32, tag="u", name="pq")
                        nc.tensor.matmul(
                            pq,
                            lhsT=sb_ssw[:, i, h, :],
                            rhs=qT[0:64, ds(512 * n2, 512)],
                            start=True, stop=True,
                        )
                        nc.scalar.activation(qw_sb[0:64, i, ds(512 * n2, 512)], pq, AF.Copy)
                for i in range(5):
                    nc.sync.dma_start(qw_sb[64:128, i, :], qw_sb[0:64, i, :])

                # B4: q-tile loop — software-pipelined emission so each
                # engine's FIFO gets independent next-tile work before the
                # dependent joins of the current tile.
                state = {}

                def stageA(t):
                    st = sstp.tile([128, 5, 1024], BF16, tag="st", name="st")
                    nc.scalar.dma_start(st, struct[t])
                    r1 = rg.tile([128, 1024], BF16, tag="r1", name="r1")
                    nc.sync.dma_start(
                        r1,
                        bass.AP(
                            tensor=dramQ.tensor,
                            offset=dramQ.offset + t * 128 * WIN + 127,
                            ap=[[WIN - 1, 128], [1, 1024]],
                        ),
                    )
                    g = rg.tile([128, NT, 128], BF16, tag="g", name="g")
                    nc.sync.dma_start(
                        g,
                        bass.AP(
                            tensor=dramK.tensor,
                            offset=dramK.offset + 128 * t + 127,
                            ap=[[WIN - 1, 128], [128 * WIN, NT], [1, 128]],
                        ),
                    )
                    pt2 = putil.tile([128, 1024], BF16, tag="u", name="pt2")
                    for j in range(NT):
                        nc.tensor.matmul(
                            pt2[:, ds(128 * j, 128)],
                            lhsT=g[:, j, :], rhs=sb_idb,
                            is_transpose=True,
                            start=(j == 0), stop=(j == NT - 1),
                        )
                    psc = psS.tile([128, 1024], F32, tag="s", name="psc")
                    nc.tensor.matmul(
                        psc[:, 0:512],
                        lhsT=qT[0:64, ds(128 * t, 128)], rhs=kT[0:64, 0:512],
                        start=True, stop=False, tile_position=(0, 0),
                    )
                    nc.tensor.matmul(
                        psc[:, 512:1024],
                        lhsT=qT[64:128, ds(128 * t, 128)],
                        rhs=kT[64:128, 512:1024],
                        start=True, stop=False, tile_position=(64, 0),
                    )
                    if use_mask:
                        for half in range(2):
                            sl = ds(512 * half, 512)
                            nc.tensor.matmul(
                                psc[:, sl], lhsT=sb_ones[0:1, :],
                                rhs=sb_mask[0:1, sl], start=False, stop=False,
                            )
                    state[t] = {"st": st, "r1": r1, "g": g, "pt2": pt2,
                                "psc": psc}

                def biasmm(t, i, half):
                    sl = ds(512 * half, 512)
                    pb = putil.tile([128, 512], F32, tag="u", name="pb")
                    rr = 0 if (i % 2 == 0) else 64
                    nc.tensor.matmul(
                        pb,
                        lhsT=qw_sb[rr:rr + 64, i, ds(128 * t, 128)],
                        rhs=kT[rr:rr + 64, sl],
                        start=True, stop=True, tile_position=(rr, 0),
                    )
                    return pb

                def stageB(t):
                    d = state[t]
                    st, r1, pt2 = d["st"], d["r1"], d["pt2"]
                    t2sb = misc.tile([128, 1024], BF16, tag="t2sb", name="t2sb")
                    if "act" == "act":
                        nc.scalar.activation(t2sb, pt2, AF.Copy)
                    else:
                        nc.vector.tensor_copy(t2sb, pt2)
                    rt = prp.tile([128, 1024], BF16, tag="rt", name="rt")
                    nc.gpsimd.tensor_tensor(rt, r1, t2sb, OP.add)
                    d["rt"] = rt
                    sums = {}
                    direct = tuple(int(c) for c in "01234")
                    evacd = tuple(i for i in range(5) if i not in direct)
                    pooli = int("9")
                    # direct set: bias MM then DVE fused (evac+absb+mult) from PSUM
                    for half in range(2):
                        sl = ds(512 * half, 512)
                        for i in direct:
                            pb = biasmm(t, i, half)
                            pr = prp.tile([128, 512], BF16, tag=f"pd{i}", name="pr")
                            nc.vector.scalar_tensor_tensor(
                                pr, in0=pb,
                                scalar=sb_absb[:, ds(3 * i + h, 1)],
                                in1=st[:, i, sl], op0=OP.add, op1=OP.mult,
                            )
                            sums[(i, half)] = pr
                    # evac set: ACT evac (+absb), then bf16 multiply (DVE/Pool)
                    ppr = {}
                    for i in evacd:
                        be = bep.tile([128, 1024], BF16, tag="be", name="be")
                        for half in range(2):
                            pb = biasmm(t, i, half)
                            nc.scalar.activation(
                                be[:, ds(512 * half, 512)], pb,
                                AF.Identity, bias=sb_absb[:, ds(3 * i + h, 1)],
                            )
                        pr = prp.tile([128, 1024], BF16, tag=f"pe{i}", name="pr2")
                        eng = nc.gpsimd if i == pooli else nc.vector
                        eng.tensor_tensor(pr, be, st[:, i, :], OP.mult)
                        ppr[i] = pr
                    nsum = int("1")
                    dl = list(direct)
                    for k in range(nsum):
                        if len(dl) >= 2:
                            a, b = dl.pop(0), dl.pop(0)
                            for half in range(2):
                                sp = prp.tile([128, 512], BF16, tag=f"sp{k}", name="sp")
                                nc.gpsimd.tensor_tensor(
                                    sp, sums[(a, half)], sums[(b, half)], OP.add)
                                sums.pop((a, half)); sums.pop((b, half))
                                sums[(f"sp{k}", half)] = sp
                    if len(evacd) >= 2:
                        s2 = prp.tile([128, 1024], BF16, tag="s2", name="s2")
                        nc.vector.tensor_tensor(s2, ppr[evacd[0]], ppr[evacd[1]], OP.add)
                        for j in evacd[2:]:
                            s2n = prp.tile([128, 1024], BF16, tag="s2", name="s2n")
                            nc.vector.tensor_tensor(s2n, s2, ppr[j], OP.add)
                            s2 = s2n
                    else:
                        s2 = ppr[evacd[0]] if evacd else None
                    d["sums"] = sums
                    d["s2"] = s2

                def stageC(t):
                    d = state[t]
                    psc, rt, sums = d["psc"], d["rt"], d["sums"]
                    for half in range(2):
                        sl = ds(512 * half, 512)
                        joins = [rt[:, sl]]
                        joins += [v for (kk, hh), v in sums.items() if hh == half]
                        if d["s2"] is not None:
                            joins.append(d["s2"][:, sl])
                        for ji, j in enumerate(joins):
                            nc.tensor.matmul(
                                psc[:, sl], lhsT=sb_idb, rhs=j,
                                start=False,
                                stop=(half == 1 and ji == len(joins) - 1),
                            )
                    probs = prob.tile([128, 1024], BF16, tag="p", name="probs")
                    rsum = misc.tile([128, 1], F32, tag="rs", name="rsum")
                    nc.scalar.activation(probs, psc, AF.Exp, accum_out=rsum)
                    d["probs"], d["rsum"] = probs, rsum

                def stageD(t):
                    d = state.pop(t)
                    probs, rsum = d["probs"], d["rsum"]
                    ptps = putil.tile([128, 1024], BF16, tag="u", name="ptps")
                    for j in range(NT):
                        nc.tensor.matmul(
                            ptps[:, ds(128 * j, 128)],
                            lhsT=probs[:, ds(128 * j, 128)], rhs=sb_idb,
                            is_transpose=True,
                            start=(j == 0), stop=(j == NT - 1),
                        )
                    ptsb = misc.tile([128, 1024], BF16, tag="ptsb", name="ptsb")
                    nc.scalar.activation(ptsb, ptps, AF.Copy)
                    ctxps = putil.tile([128, 64], F32, tag="u", name="ctxps")
                    for j in range(NT):
                        nc.tensor.matmul(
                            ctxps,
                            lhsT=ptsb[:, ds(128 * j, 128)],
                            rhs=vsb[:, j, ds(64 * h, 64)],
                            start=(j == 0), stop=(j == NT - 1),
                        )
                    rec = misc.tile([128, 1], F32, tag="rc", name="rec")
                    nc.vector.reciprocal(rec, rsum)
                    cn = misc.tile([128, 64], F32, tag="cn", name="cn")
                    nc.vector.tensor_scalar_mul(cn, ctxps, rec)
                    nc.sync.dma_start(out[t, :, ds(64 * h, 64)], cn)

                # pipelined emission: [D(k-2), A(k), B(k), C(k-1)]
                for k in range(NT + 2):
                    if 0 <= k - 2:
                        stageD(k - 2)
                    if k < NT:
                        stageA(k)
                        stageB(k)
                    if 0 <= k - 1 < NT:
                        stageC(k - 1)

    nc.compile()
    return nc, out


_PROGRAM_CACHE = {}


def kernel(**inputs):
    hs = np.asarray(inputs["hidden_states"], np.float32)
    mask = np.asarray(inputs["attention_mask"], np.float32)
    struct = np.asarray(inputs["struct_matrix"], np.float32)
    Wq = np.asarray(inputs["Wq"], np.float32)
    bq = np.asarray(inputs["bq"], np.float32)
    Wk = np.asarray(inputs["Wk"], np.float32)
    bk = np.asarray(inputs["bk"], np.float32)
    Wv = np.asarray(inputs["Wv"], np.float32)
    bv = np.asarray(inputs["bv"], np.float32)
    E = np.asarray(inputs["dist_emb"], np.float32)
    ssw = np.asarray(inputs["ssan_w"], np.float32)
    absb = np.asarray(inputs["abs_bias"], np.float32)

    bf = ml_dtypes.bfloat16
    use_mask = bool(np.any(mask))
    use_pbias = bool(np.any(bq) or np.any(bk) or np.any(bv))

    key = (use_mask, use_pbias)
    if key not in _PROGRAM_CACHE:
        _PROGRAM_CACHE[key] = build_program(use_mask, use_pbias)
    nc, _ = _PROGRAM_CACHE[key]

    Epad = np.concatenate([E, np.zeros((1, DH), np.float32)])
    Erev = np.concatenate([E[::-1], np.zeros((1, DH), np.float32)])
    ert_half = np.ascontiguousarray(Erev.T)
    et_half = np.ascontiguousarray(Epad.T) / 8.0
    ert_np = np.concatenate([ert_half, ert_half], 0).astype(bf)
    et_np = np.concatenate([et_half, et_half], 0).astype(bf)
    idb_np = np.eye(128, dtype=np.float32).astype(bf)
    idf_np = np.eye(128, dtype=np.float32)

    in_maps = []
    for c in range(8):
        b = c // 4
        h0 = 3 * (c % 4)
        hsT = hs[b].T
        m = {
            "hsT": np.ascontiguousarray(
                hsT.reshape(NCHUNK, 128, 1024).transpose(1, 0, 2)
            ).astype(bf),
            "wq": np.ascontiguousarray(
                (Wq[:, h0 * 64:(h0 + 3) * 64] / 8.0)
                .reshape(NCHUNK, 128, 192).transpose(1, 0, 2)
            ).astype(bf),
            "wk": np.ascontiguousarray(
                Wk[:, h0 * 64:(h0 + 3) * 64]
                .reshape(NCHUNK, 128, 192).transpose(1, 0, 2)
            ).astype(bf),
            "wv": np.ascontiguousarray(
                Wv[:, h0 * 64:(h0 + 3) * 64]
                .reshape(NCHUNK, 128, 192).transpose(1, 0, 2)
            ).astype(bf),
            "ert": ert_np,
            "et": et_np,
            "ssw": np.ascontiguousarray(
                (ssw[:, h0:h0 + 3] * 8.0).transpose(2, 0, 1, 3)
            ).astype(bf),
            "struct": np.ascontiguousarray(
                struct[:, b, 0].reshape(5, NT, 128, 1024).transpose(1, 2, 0, 3)
            ).astype(bf),
            "absb": np.concatenate(
                [absb[:, h0:h0 + 3].reshape(1, 15),
                 np.zeros((1, 1), np.float32)], 1
            ),
            "idb": idb_np,
            "idf": idf_np,
        }
        if use_mask:
            m["maskv"] = mask[b, 0, 0].reshape(1, 1024).astype(bf)
            m["onesv"] = np.ones((1, 128), np.float32).astype(bf)
        if use_pbias:
            m["bqv"] = (bq[h0 * 64:(h0 + 3) * 64] / 8.0).reshape(1, 192).astype(bf)
            m["bkv"] = bk[h0 * 64:(h0 + 3) * 64].reshape(1, 192).astype(bf)
            m["bvv"] = bv[h0 * 64:(h0 + 3) * 64].reshape(1, 192).astype(bf)
            m["onesL"] = np.ones((1, 1024), np.float32).astype(bf)
        in_maps.append(m)

    res = run_bass_kernel_spmd(nc, in_maps, core_ids=list(range(8)))
    outs = [r["out"] for r in res.results]

    full = np.zeros((B, L, D), np.float32)
    for c in range(8):
        b = c // 4
        h0 = 3 * (c % 4)
        o = np.asarray(outs[c], np.float32).reshape(L, 192)
        for j in range(3):
            full[b, :, (h0 + j) * 64:(h0 + j + 1) * 64] = o[:, j * 64:(j + 1) * 64]
    return full

